# revision 15
# baseline (speedup 1.0000x reference)
"""Cross-attention Bass kernel for Trainium2, 8 NeuronCores.

Sharding (hardcoded for B=4, Sq=Skv=2048, 16 heads, dim_head=64):
  core = 2*b + h  (b in 0..3 batches, h in 0..1 head-halves)
  - data parallel over batch B (4-way)
  - tensor parallel over heads (2-way): each core owns 8 heads = 512 of the
    1024 inner columns (to_q/k/v column-parallel, to_out row-parallel)
  - to_out partial sums are combined with an on-device ReduceScatter over
    core pairs {2b, 2b+1}; each core returns half of the rows of out[b].

The host pre-transposes x/context per batch (xT = x[b].T) so the kernel's
matmuls get the contraction dim on partitions without on-chip transposes.

All activation layouts inside the kernel are "transposed" (feature-major):
  qT/kT: [inner=128*pair, seq]; v: [seq, inner]; attention is computed as
  simT[j, s] = k_h @ q_h^T so the softmax denominator is accumulated with a
  ones-column matmul and applied after the PV product.
"""

import sys

for _p in ("/opt/trn_rl_repo",):
    if _p not in sys.path:
        sys.path.insert(0, _p)

from contextlib import ExitStack

import numpy as np

import concourse.bass as bass
import concourse.mybir as mybir
import concourse.tile as tile
from concourse import bacc
from concourse.bass import ts

F32 = mybir.dt.float32
F32R = mybir.dt.float32r

# full-size problem constants
HEADS = 16
DIM_HEAD = 64
QUERY_DIM = 1024
CONTEXT_DIM = 768
INNER = HEADS * DIM_HEAD  # 1024
B_FULL, SQ_FULL, O_FULL = 4, 2048, 1024
N_CORES = 8


def build_nc(S=2048, C=1024, CK=768, I=512, O=1024, SC=512, n_cores=8,
             use_f32r=True):
    """Build the per-core SPMD Bass program.

    S: q/kv sequence length, C: query dim, CK: context dim,
    I: per-core inner size (heads_per_core * 64), O: output dim,
    SC: s-chunk width used as matmul moving size (<=512 for fp32).
    """
    D = 64
    n_pairs = I // 128            # head pairs per core
    assert n_pairs % 2 == 0 or n_pairs == 2, "pp loop needs pairs in groups of 2"
    CT, CKT = C // 128, CK // 128
    NSC = S // SC                 # q chunks
    NJ = S // 128                 # kv blocks
    NSB = SC // 128               # s-blocks per chunk
    NOC = O // 512                # out column chunks
    NH = I // 64                  # heads per core
    scale = D ** -0.5
    groups = [[2 * i, 2 * i + 1] for i in range(n_cores // 2)]

    MMDT = F32R if use_f32r else F32   # matmul operand dtype (weights/acts)
    BF16 = mybir.dt.bfloat16 if use_f32r else F32  # attention-prob path

    nc = bacc.Bacc("TRN2", target_bir_lowering=False, debug=False,
                   num_devices=n_cores)

    xT = nc.dram_tensor("xT", [C, S], MMDT, kind="ExternalInput").ap()
    ctxT = nc.dram_tensor("ctxT", [CK, S], MMDT, kind="ExternalInput").ap()
    wq = nc.dram_tensor("wq", [C, I], MMDT, kind="ExternalInput").ap()
    wk = nc.dram_tensor("wk", [CK, I], MMDT, kind="ExternalInput").ap()
    wv = nc.dram_tensor("wv", [CK, I], MMDT, kind="ExternalInput").ap()
    wo = nc.dram_tensor("wo", [I, O], MMDT, kind="ExternalInput").ap()
    bo = nc.dram_tensor("bo", [1, O], F32, kind="ExternalInput").ap()
    out_ext = nc.dram_tensor("out", [S // 2, O], F32, kind="ExternalOutput").ap()

    with tile.TileContext(nc) as tc, ExitStack() as stk:
        dram = stk.enter_context(tc.tile_pool(name="dram", bufs=1, space="DRAM"))
        rs_in = dram.tile([S, O], F32, tag="rs_in")
        rs_out = [
            dram.tile([SC // 2, O], F32, tag=f"rs_out{i}", name=f"rs_out{i}")
            for i in range(NSC)
        ]

        persist = stk.enter_context(tc.tile_pool(name="persist", bufs=1))
        qT = [persist.tile([128, S], MMDT, tag=f"qT{p}", name=f"qT{p}")
              for p in range(n_pairs)]
        kT = [persist.tile([128, S], MMDT, tag=f"kT{p}", name=f"kT{p}")
              for p in range(n_pairs)]
        # v augmented with a per-head ones column (65 cols/head): the PV
        # matmul then emits the softmax denominator as psum row 64.
        v_sb = [persist.tile([128, NH * 65], MMDT, tag=f"v{j}", name=f"v{j}")
                for j in range(NJ)]
        wo_sb = [persist.tile([128, O], MMDT, tag=f"wo{p}", name=f"wo{p}")
                 for p in range(n_pairs)]
        bias_sb = persist.tile([128, O], F32, tag="bias", name="bias_sb")
        ones_f32 = persist.tile([128, NH], F32, tag="ones_f", name="ones_f32")

        nc.sync.dma_start(out=bias_sb[:], in_=bo.to_broadcast((128, O)))
        nc.vector.memset(ones_f32[:], 1.0)
        for p in range(n_pairs):
            nc.sync.dma_start(out=wo_sb[p][:], in_=wo[ts(p, 128), :])

        # ---------------- projections: k & v from ctxT ----------------
        with ExitStack() as pstk:
            wpool = pstk.enter_context(tc.tile_pool(name="wkv", bufs=1))
            inp = pstk.enter_context(tc.tile_pool(name="inkv", bufs=2))
            psum = pstk.enter_context(
                tc.tile_pool(name="pskv", bufs=4, space="PSUM"))
            wk_sb = [wpool.tile([128, I], MMDT, tag=f"wk{c}", name=f"wk{c}")
                     for c in range(CKT)]
            wv_sb = [wpool.tile([128, I], MMDT, tag=f"wv{c}", name=f"wv{c}")
                     for c in range(CKT)]
            for c in range(CKT):
                nc.sync.dma_start(out=wk_sb[c][:], in_=wk[ts(c, 128), :])
                nc.sync.dma_start(out=wv_sb[c][:], in_=wv[ts(c, 128), :])
            for sc in range(NSC):
                chunk = [inp.tile([128, SC], MMDT, tag=f"ckv{c}", name=f"ckv{c}")
                         for c in range(CKT)]
                for c in range(CKT):
                    nc.sync.dma_start(out=chunk[c][:],
                                      in_=ctxT[ts(c, 128), ts(sc, SC)])
                # kT[p][:, sc*SC:...] = (wk[:, p-slab].T @ ctxT[:, chunk])
                for p in range(n_pairs):
                    acc = psum.tile([128, SC], F32, tag="pkv", name="acc_kv")
                    for c in range(CKT):
                        nc.tensor.matmul(
                            acc[:], wk_sb[c][:, ts(p, 128)], chunk[c][:],
                            start=(c == 0), stop=(c == CKT - 1))
                    nc.vector.tensor_copy(kT[p][:, ts(sc, SC)], acc[:])
                # v rows for this chunk: v[jb] = ctxT_chunk.T @ wv
                IC = min(I, 512)
                for jb in range(NSB):
                    j = sc * NSB + jb
                    for ic in range(I // IC):
                        acc = psum.tile([128, IC], F32, tag="pkv",
                                        name="acc_v")
                        for c in range(CKT):
                            nc.tensor.matmul(
                                acc[:], chunk[c][:, ts(jb, 128)],
                                wv_sb[c][:, ts(ic, IC)],
                                start=(c == 0), stop=(c == CKT - 1))
                        nh_c = IC // 64  # heads covered by this chunk
                        v_view = v_sb[j][:].rearrange(
                            "p (h e) -> p h e", e=65)
                        nc.vector.tensor_copy(
                            v_view[:, ic * nh_c:(ic + 1) * nh_c, 0:64],
                            acc[:].rearrange("p (h d) -> p h d", d=64))
                        nc.vector.tensor_copy(
                            v_view[:, ic * nh_c:(ic + 1) * nh_c, 64:65],
                            ones_f32[:, 0:nh_c].rearrange(
                                "p (h o) -> p h o", o=1))

        # ---------------- projections: q from xT ----------------
        with ExitStack() as pstk:
            wpool = pstk.enter_context(tc.tile_pool(name="wq", bufs=1))
            inp = pstk.enter_context(tc.tile_pool(name="inq", bufs=2))
            psum = pstk.enter_context(
                tc.tile_pool(name="psq", bufs=4, space="PSUM"))
            wq_sb = [wpool.tile([128, I], MMDT, tag=f"wq{c}", name=f"wq{c}")
                     for c in range(CT)]
            for c in range(CT):
                nc.sync.dma_start(out=wq_sb[c][:], in_=wq[ts(c, 128), :])
            for sc in range(NSC):
                chunk = [inp.tile([128, SC], MMDT, tag=f"cq{c}", name=f"cq{c}")
                         for c in range(CT)]
                for c in range(CT):
                    nc.sync.dma_start(out=chunk[c][:],
                                      in_=xT[ts(c, 128), ts(sc, SC)])
                for p in range(n_pairs):
                    acc = psum.tile([128, SC], F32, tag="pq", name="acc_q")
                    for c in range(CT):
                        nc.tensor.matmul(
                            acc[:], wq_sb[c][:, ts(p, 128)], chunk[c][:],
                            start=(c == 0), stop=(c == CT - 1))
                    nc.vector.tensor_copy(qT[p][:, ts(sc, SC)], acc[:])

        # ---------------- attention + output projection ----------------
        with ExitStack() as astk:
            ps_sim = astk.enter_context(
                tc.tile_pool(name="ps_sim", bufs=2, space="PSUM"))
            ps_oT = astk.enter_context(
                tc.tile_pool(name="ps_oT", bufs=4, space="PSUM"))
            epool = astk.enter_context(tc.tile_pool(name="epool", bufs=4))
            opool = astk.enter_context(tc.tile_pool(name="opool", bufs=8))
            npool = astk.enter_context(tc.tile_pool(name="npool", bufs=4))
            outp = astk.enter_context(tc.tile_pool(name="outp", bufs=4))

            for sc in range(NSC):
                oT_chunk = {}
                for p in range(n_pairs):
                    # per-head PV accumulators: rows 0..63 = oT, row 64 = sums
                    oT_ps = [ps_oT.tile([128, SC], F32, tag="oT",
                                        name=f"oT_ps{h}") for h in range(2)]
                    v_view = [v_sb[j][:].rearrange("p (h e) -> p h e", e=65)
                              for j in range(NJ)]
                    for j in range(NJ):
                        sim = ps_sim.tile([128, 2 * SC], F32, tag="sim",
                                          name="sim")
                        for h in range(2):  # head within pair
                            nc.tensor.matmul(
                                sim[:, ts(h, SC)],
                                kT[p][ts(h, 64), ts(j, 128)],
                                qT[p][ts(h, 64), ts(sc, SC)],
                                start=True, stop=True)
                        e = epool.tile([128, 2 * SC], MMDT, tag="E", name="E")
                        nc.scalar.activation(
                            e[:], sim[:],
                            mybir.ActivationFunctionType.Exp, scale=scale)
                        first, last = (j == 0), (j == NJ - 1)
                        for h in range(2):
                            nc.tensor.matmul(
                                oT_ps[h][0:65, :],
                                v_view[j][:, 2 * p + h, :],
                                e[:, ts(h, SC)],
                                start=first, stop=last)
                    # normalize: oT_h /= sums_h (broadcast 1/sums via DRAM)
                    o_sb = opool.tile([128, SC], MMDT, tag="oT_sb",
                                      name="oT_sb")
                    for h in range(2):
                        rec = npool.tile([1, SC], F32, tag="rec", name="rec")
                        nc.vector.reciprocal(rec[:], oT_ps[h][64:65, :])
                        rec_d = dram.tile([1, SC], F32, tag="rec_d",
                                          bufs=8, name="rec_d")
                        nc.sync.dma_start(out=rec_d[:], in_=rec[:])
                        bcast = npool.tile([64, SC], F32, tag="bcast",
                                           name="bcast")
                        nc.sync.dma_start(
                            out=bcast[:],
                            in_=rec_d[0:1, :].to_broadcast((64, SC)))
                        if h == 0:
                            nc.vector.tensor_mul(o_sb[0:64, :],
                                                 oT_ps[h][0:64, :], bcast[:])
                        else:
                            # DVE lanes are partition-locked; normalize in
                            # place then DMA-shift rows into the pair slab.
                            tb = npool.tile([64, SC], MMDT, tag="tb",
                                            name="tb")
                            nc.vector.tensor_mul(tb[:], oT_ps[h][0:64, :],
                                                 bcast[:])
                            nc.sync.dma_start(out=o_sb[64:128, :], in_=tb[:])
                    oT_chunk[p] = o_sb
                # out projection for this chunk + bias
                for sb in range(NSB):
                    for oc in range(NOC):
                        acc = ps_sim.tile([128, 512], F32, tag="sim",
                                          name="acc_o")
                        for p in range(n_pairs):
                            nc.tensor.matmul(
                                acc[:], oT_chunk[p][:, ts(sb, 128)],
                                wo_sb[p][:, ts(oc, 512)],
                                start=(p == 0), stop=(p == n_pairs - 1))
                        o_out = outp.tile([128, 512], F32, tag="o_out",
                                          name="o_out")
                        nc.vector.tensor_add(o_out[:], acc[:],
                                             bias_sb[:, ts(oc, 512)])
                        nc.sync.dma_start(
                            out=rs_in[sc * SC + sb * 128:
                                      sc * SC + sb * 128 + 128, ts(oc, 512)],
                            in_=o_out[:])
                nc.gpsimd.collective_compute(
                    "ReduceScatter", mybir.AluOpType.add,
                    replica_groups=groups,
                    ins=[rs_in[ts(sc, SC), :]],
                    outs=[rs_out[sc][:]])
                nc.sync.dma_start(out=out_ext[ts(sc, SC // 2), :],
                                  in_=rs_out[sc][:])

    nc.compile()
    return nc


# ---------------------------------------------------------------------------
# host-side sharding / unsharding
# ---------------------------------------------------------------------------

def make_in_maps(x, context, w_q, w_k, w_v, w_o, b_o, n_cores=N_CORES):
    x = np.asarray(x, dtype=np.float32)
    context = np.asarray(context, dtype=np.float32)
    w_q = np.asarray(w_q, dtype=np.float32)
    w_k = np.asarray(w_k, dtype=np.float32)
    w_v = np.asarray(w_v, dtype=np.float32)
    w_o = np.asarray(w_o, dtype=np.float32)
    b_o = np.asarray(b_o, dtype=np.float32)
    inner = w_q.shape[1]
    ih = inner // 2  # per-core inner half
    zeros_b = np.zeros_like(b_o)
    in_maps = []
    for core in range(n_cores):
        b, hh = core // 2, core % 2
        i0 = hh * ih
        in_maps.append({
            "xT": np.ascontiguousarray(x[b].T),
            "ctxT": np.ascontiguousarray(context[b].T),
            "wq": np.ascontiguousarray(w_q[:, i0:i0 + ih]),
            "wk": np.ascontiguousarray(w_k[:, i0:i0 + ih]),
            "wv": np.ascontiguousarray(w_v[:, i0:i0 + ih]),
            "wo": np.ascontiguousarray(w_o[i0:i0 + ih, :]),
            "bo": (b_o if hh == 0 else zeros_b).reshape(1, -1).copy(),
        })
    return in_maps


def gather_out(results, S, O, SC, n_cores=N_CORES):
    """Assemble full output from per-core ReduceScatter shards."""
    B = n_cores // 2
    out = np.empty((B, S, O), dtype=np.float32)
    nsc = S // SC
    half = SC // 2
    for core in range(n_cores):
        b, hh = core // 2, core % 2
        res = results[core]["out"]  # [S//2, O]
        for c in range(nsc):
            rows = res[c * half:(c + 1) * half]
            out[b, c * SC + hh * half: c * SC + (hh + 1) * half, :] = rows
    return out


_NC_CACHE = {}


def _get_nc():
    if "full" not in _NC_CACHE:
        _NC_CACHE["full"] = build_nc()
    return _NC_CACHE["full"]


def kernel(x, context, w_q, w_k, w_v, w_o, b_o):
    from concourse.bass_utils import run_bass_kernel_spmd

    nc = _get_nc()
    in_maps = make_in_maps(x, context, w_q, w_k, w_v, w_o, b_o)
    res = run_bass_kernel_spmd(nc, in_maps, list(range(N_CORES)))
    return gather_out(res.results, SQ_FULL, O_FULL, 512)


# revision 16
# speedup vs baseline: 1.0009x; 1.0009x over previous
"""Cross-attention Bass kernel for Trainium2, 8 NeuronCores.

Sharding (hardcoded for B=4, Sq=Skv=2048, 16 heads, dim_head=64):
  core = 2*b + h  (b in 0..3 batches, h in 0..1 head-halves)
  - data parallel over batch B (4-way)
  - tensor parallel over heads (2-way): each core owns 8 heads = 512 of the
    1024 inner columns (to_q/k/v column-parallel, to_out row-parallel)
  - to_out partial sums are combined with an on-device ReduceScatter over
    core pairs {2b, 2b+1}; each core returns half of the rows of out[b].

The host pre-transposes x/context per batch (xT = x[b].T) so the kernel's
matmuls get the contraction dim on partitions without on-chip transposes.

All activation layouts inside the kernel are "transposed" (feature-major):
  qT/kT: [inner=128*pair, seq]; v: [seq, inner]; attention is computed as
  simT[j, s] = k_h @ q_h^T so the softmax denominator is accumulated with a
  ones-column matmul and applied after the PV product.
"""

import sys

for _p in ("/opt/trn_rl_repo",):
    if _p not in sys.path:
        sys.path.insert(0, _p)

from contextlib import ExitStack

import numpy as np

import concourse.bass as bass
import concourse.mybir as mybir
import concourse.tile as tile
from concourse import bacc
from concourse.bass import ts

F32 = mybir.dt.float32
F32R = mybir.dt.float32r

# full-size problem constants
HEADS = 16
DIM_HEAD = 64
QUERY_DIM = 1024
CONTEXT_DIM = 768
INNER = HEADS * DIM_HEAD  # 1024
B_FULL, SQ_FULL, O_FULL = 4, 2048, 1024
N_CORES = 8


def build_nc(S=2048, C=1024, CK=768, I=512, O=1024, SC=512, n_cores=8,
             use_f32r=True):
    """Build the per-core SPMD Bass program.

    S: q/kv sequence length, C: query dim, CK: context dim,
    I: per-core inner size (heads_per_core * 64), O: output dim,
    SC: s-chunk width used as matmul moving size (<=512 for fp32).
    """
    D = 64
    n_pairs = I // 128            # head pairs per core
    assert n_pairs % 2 == 0 or n_pairs == 2, "pp loop needs pairs in groups of 2"
    CT, CKT = C // 128, CK // 128
    NSC = S // SC                 # q chunks
    NJ = S // 128                 # kv blocks
    NSB = SC // 128               # s-blocks per chunk
    NOC = O // 512                # out column chunks
    NH = I // 64                  # heads per core
    scale = D ** -0.5
    groups = [[2 * i, 2 * i + 1] for i in range(n_cores // 2)]

    MMDT = F32R if use_f32r else F32   # matmul operand dtype (weights/acts)
    BF16 = mybir.dt.bfloat16 if use_f32r else F32  # attention-prob path

    nc = bacc.Bacc("TRN2", target_bir_lowering=False, debug=False,
                   num_devices=n_cores)

    xT = nc.dram_tensor("xT", [C, S], MMDT, kind="ExternalInput").ap()
    ctxT = nc.dram_tensor("ctxT", [CK, S], MMDT, kind="ExternalInput").ap()
    wq = nc.dram_tensor("wq", [C, I], MMDT, kind="ExternalInput").ap()
    wk = nc.dram_tensor("wk", [CK, I], MMDT, kind="ExternalInput").ap()
    wv = nc.dram_tensor("wv", [CK, I], MMDT, kind="ExternalInput").ap()
    wo = nc.dram_tensor("wo", [I, O], MMDT, kind="ExternalInput").ap()
    bo = nc.dram_tensor("bo", [1, O], F32, kind="ExternalInput").ap()
    out_ext = nc.dram_tensor("out", [S // 2, O], F32, kind="ExternalOutput").ap()

    with tile.TileContext(nc) as tc, ExitStack() as stk:
        dram = stk.enter_context(tc.tile_pool(name="dram", bufs=1, space="DRAM"))
        rs_in = dram.tile([S, O], F32, tag="rs_in")
        rs_out = [
            dram.tile([SC // 2, O], F32, tag=f"rs_out{i}", name=f"rs_out{i}")
            for i in range(NSC)
        ]

        persist = stk.enter_context(tc.tile_pool(name="persist", bufs=1))
        qT = [persist.tile([128, S], MMDT, tag=f"qT{p}", name=f"qT{p}")
              for p in range(n_pairs)]
        kT = [persist.tile([128, S], MMDT, tag=f"kT{p}", name=f"kT{p}")
              for p in range(n_pairs)]
        # v augmented with a per-head ones column (65 cols/head): the PV
        # matmul then emits the softmax denominator as psum row 64.
        v_sb = [persist.tile([128, NH * 65], MMDT, tag=f"v{j}", name=f"v{j}")
                for j in range(NJ)]
        wo_sb = [persist.tile([128, O], MMDT, tag=f"wo{p}", name=f"wo{p}")
                 for p in range(n_pairs)]
        bias_sb = persist.tile([128, O], F32, tag="bias", name="bias_sb")
        ones_f32 = persist.tile([128, NH], F32, tag="ones_f", name="ones_f32")

        nc.sync.dma_start(out=bias_sb[:], in_=bo.to_broadcast((128, O)))
        nc.vector.memset(ones_f32[:], 1.0)
        for p in range(n_pairs):
            nc.sync.dma_start(out=wo_sb[p][:], in_=wo[ts(p, 128), :])

        # ---------------- projections: k & v from ctxT ----------------
        with ExitStack() as pstk:
            wpool = pstk.enter_context(tc.tile_pool(name="wkv", bufs=1))
            inp = pstk.enter_context(tc.tile_pool(name="inkv", bufs=2))
            psum = pstk.enter_context(
                tc.tile_pool(name="pskv", bufs=4, space="PSUM"))
            wk_sb = [wpool.tile([128, I], MMDT, tag=f"wk{c}", name=f"wk{c}")
                     for c in range(CKT)]
            wv_sb = [wpool.tile([128, I], MMDT, tag=f"wv{c}", name=f"wv{c}")
                     for c in range(CKT)]
            for c in range(CKT):
                nc.sync.dma_start(out=wk_sb[c][:], in_=wk[ts(c, 128), :])
                nc.sync.dma_start(out=wv_sb[c][:], in_=wv[ts(c, 128), :])
            for sc in range(NSC):
                chunk = [inp.tile([128, SC], MMDT, tag=f"ckv{c}", name=f"ckv{c}")
                         for c in range(CKT)]
                for c in range(CKT):
                    nc.sync.dma_start(out=chunk[c][:],
                                      in_=ctxT[ts(c, 128), ts(sc, SC)])
                # kT[p][:, sc*SC:...] = (wk[:, p-slab].T @ ctxT[:, chunk])
                for p in range(n_pairs):
                    acc = psum.tile([128, SC], F32, tag="pkv", name="acc_kv")
                    for c in range(CKT):
                        nc.tensor.matmul(
                            acc[:], wk_sb[c][:, ts(p, 128)], chunk[c][:],
                            start=(c == 0), stop=(c == CKT - 1))
                    nc.vector.tensor_copy(kT[p][:, ts(sc, SC)], acc[:])
                # v rows for this chunk: v[jb] = ctxT_chunk.T @ wv
                IC = min(I, 512)
                for jb in range(NSB):
                    j = sc * NSB + jb
                    for ic in range(I // IC):
                        acc = psum.tile([128, IC], F32, tag="pkv",
                                        name="acc_v")
                        for c in range(CKT):
                            nc.tensor.matmul(
                                acc[:], chunk[c][:, ts(jb, 128)],
                                wv_sb[c][:, ts(ic, IC)],
                                start=(c == 0), stop=(c == CKT - 1))
                        nh_c = IC // 64  # heads covered by this chunk
                        v_view = v_sb[j][:].rearrange(
                            "p (h e) -> p h e", e=65)
                        nc.vector.tensor_copy(
                            v_view[:, ic * nh_c:(ic + 1) * nh_c, 0:64],
                            acc[:].rearrange("p (h d) -> p h d", d=64))
                        nc.vector.tensor_copy(
                            v_view[:, ic * nh_c:(ic + 1) * nh_c, 64:65],
                            ones_f32[:, 0:nh_c].rearrange(
                                "p (h o) -> p h o", o=1))

        # ---------------- projections: q from xT ----------------
        with ExitStack() as pstk:
            wpool = pstk.enter_context(tc.tile_pool(name="wq", bufs=1))
            inp = pstk.enter_context(tc.tile_pool(name="inq", bufs=2))
            psum = pstk.enter_context(
                tc.tile_pool(name="psq", bufs=4, space="PSUM"))
            wq_sb = [wpool.tile([128, I], MMDT, tag=f"wq{c}", name=f"wq{c}")
                     for c in range(CT)]
            for c in range(CT):
                nc.sync.dma_start(out=wq_sb[c][:], in_=wq[ts(c, 128), :])
            for sc in range(NSC):
                chunk = [inp.tile([128, SC], MMDT, tag=f"cq{c}", name=f"cq{c}")
                         for c in range(CT)]
                for c in range(CT):
                    nc.sync.dma_start(out=chunk[c][:],
                                      in_=xT[ts(c, 128), ts(sc, SC)])
                for p in range(n_pairs):
                    acc = psum.tile([128, SC], F32, tag="pq", name="acc_q")
                    for c in range(CT):
                        nc.tensor.matmul(
                            acc[:], wq_sb[c][:, ts(p, 128)], chunk[c][:],
                            start=(c == 0), stop=(c == CT - 1))
                    nc.vector.tensor_copy(qT[p][:, ts(sc, SC)], acc[:])

        # ---------------- attention + output projection ----------------
        with ExitStack() as astk:
            ps_sim = astk.enter_context(
                tc.tile_pool(name="ps_sim", bufs=2, space="PSUM"))
            ps_oT = astk.enter_context(
                tc.tile_pool(name="ps_oT", bufs=4, space="PSUM"))
            epool = astk.enter_context(tc.tile_pool(name="epool", bufs=4))
            opool = astk.enter_context(tc.tile_pool(name="opool", bufs=8))
            npool = astk.enter_context(tc.tile_pool(name="npool", bufs=4))
            outp = astk.enter_context(tc.tile_pool(name="outp", bufs=4))

            v_view = [v_sb[j][:].rearrange("p (h e) -> p h e", e=65)
                      for j in range(NJ)]
            SPP = SC // 128  # per-partition cols when the 1/sums row is
            #                  spread across 128 partitions for reciprocal

            def attention_chunk(sc):
                oT_chunk = {}
                for p in range(n_pairs):
                    # per-head PV accumulators: rows 0..63 = oT, row 64 = sums
                    oT_ps = [ps_oT.tile([128, SC], F32, tag="oT",
                                        name=f"oT_ps{h}") for h in range(2)]
                    for j in range(NJ):
                        sim = ps_sim.tile([128, 2 * SC], F32, tag="sim",
                                          name="sim")
                        for h in range(2):  # head within pair
                            nc.tensor.matmul(
                                sim[:, ts(h, SC)],
                                kT[p][ts(h, 64), ts(j, 128)],
                                qT[p][ts(h, 64), ts(sc, SC)],
                                start=True, stop=True)
                        e = epool.tile([128, 2 * SC], MMDT, tag="E", name="E")
                        nc.scalar.activation(
                            e[:], sim[:],
                            mybir.ActivationFunctionType.Exp, scale=scale)
                        first, last = (j == 0), (j == NJ - 1)
                        for h in range(2):
                            nc.tensor.matmul(
                                oT_ps[h][0:65, :],
                                v_view[j][:, 2 * p + h, :],
                                e[:, ts(h, SC)],
                                start=first, stop=last)
                    # normalize: oT_h /= sums_h. The 1/x runs with the row
                    # spread over 128 partitions (DVE reciprocal is ~8
                    # cycles/elem/lane), then is broadcast back via DRAM.
                    o_sb = opool.tile([128, SC], MMDT, tag="oT_sb",
                                      name="oT_sb")
                    for h in range(2):
                        srow = npool.tile([1, SC], F32, tag="srow",
                                          name="srow")
                        nc.vector.tensor_copy(srow[:], oT_ps[h][64:65, :])
                        sum_d = dram.tile([1, SC], F32, tag="sum_d",
                                          bufs=8, name="sum_d")
                        nc.sync.dma_start(out=sum_d[:], in_=srow[:])
                        spp = npool.tile([128, SPP], F32, tag="spp",
                                         name="spp")
                        nc.sync.dma_start(
                            out=spp[:],
                            in_=sum_d[0:1, :].rearrange(
                                "o (p f) -> (o p) f", p=128))
                        rpp = npool.tile([128, SPP], F32, tag="rpp",
                                         name="rpp")
                        nc.vector.reciprocal(rpp[:], spp[:])
                        rec_d = dram.tile([1, SC], F32, tag="rec_d",
                                          bufs=8, name="rec_d")
                        nc.sync.dma_start(
                            out=rec_d[0:1, :].rearrange(
                                "o (p f) -> (o p) f", p=128),
                            in_=rpp[:])
                        bcast = npool.tile([64, SC], F32, tag="bcast",
                                           name="bcast")
                        nc.sync.dma_start(
                            out=bcast[:],
                            in_=rec_d[0:1, :].to_broadcast((64, SC)))
                        if h == 0:
                            nc.vector.tensor_mul(o_sb[0:64, :],
                                                 oT_ps[h][0:64, :], bcast[:])
                        else:
                            # DVE lanes are partition-locked; normalize in
                            # place then DMA-shift rows into the pair slab.
                            tb = npool.tile([64, SC], MMDT, tag="tb",
                                            name="tb")
                            nc.vector.tensor_mul(tb[:], oT_ps[h][0:64, :],
                                                 bcast[:])
                            nc.sync.dma_start(out=o_sb[64:128, :], in_=tb[:])
                    oT_chunk[p] = o_sb
                return oT_chunk

            def outproj_chunk(sc, oT_chunk):
                for sb in range(NSB):
                    for oc in range(NOC):
                        acc = ps_sim.tile([128, 512], F32, tag="sim",
                                          name="acc_o")
                        for p in range(n_pairs):
                            nc.tensor.matmul(
                                acc[:], oT_chunk[p][:, ts(sb, 128)],
                                wo_sb[p][:, ts(oc, 512)],
                                start=(p == 0), stop=(p == n_pairs - 1))
                        o_out = outp.tile([128, 512], F32, tag="o_out",
                                          name="o_out")
                        nc.vector.tensor_add(o_out[:], acc[:],
                                             bias_sb[:, ts(oc, 512)])
                        nc.sync.dma_start(
                            out=rs_in[sc * SC + sb * 128:
                                      sc * SC + sb * 128 + 128, ts(oc, 512)],
                            in_=o_out[:])
                nc.gpsimd.collective_compute(
                    "ReduceScatter", mybir.AluOpType.add,
                    replica_groups=groups,
                    ins=[rs_in[ts(sc, SC), :]],
                    outs=[rs_out[sc][:]])
                nc.sync.dma_start(out=out_ext[ts(sc, SC // 2), :],
                                  in_=rs_out[sc][:])

            # software pipeline: emit out-projection of chunk sc-1 after
            # the attention of chunk sc so the PE never stalls on the
            # normalization chain.
            prev = None
            for sc in range(NSC):
                oT_chunk = attention_chunk(sc)
                if prev is not None:
                    outproj_chunk(*prev)
                prev = (sc, oT_chunk)
            outproj_chunk(*prev)

    nc.compile()
    return nc


# ---------------------------------------------------------------------------
# host-side sharding / unsharding
# ---------------------------------------------------------------------------

def make_in_maps(x, context, w_q, w_k, w_v, w_o, b_o, n_cores=N_CORES):
    x = np.asarray(x, dtype=np.float32)
    context = np.asarray(context, dtype=np.float32)
    w_q = np.asarray(w_q, dtype=np.float32)
    w_k = np.asarray(w_k, dtype=np.float32)
    w_v = np.asarray(w_v, dtype=np.float32)
    w_o = np.asarray(w_o, dtype=np.float32)
    b_o = np.asarray(b_o, dtype=np.float32)
    inner = w_q.shape[1]
    ih = inner // 2  # per-core inner half
    zeros_b = np.zeros_like(b_o)
    in_maps = []
    for core in range(n_cores):
        b, hh = core // 2, core % 2
        i0 = hh * ih
        in_maps.append({
            "xT": np.ascontiguousarray(x[b].T),
            "ctxT": np.ascontiguousarray(context[b].T),
            "wq": np.ascontiguousarray(w_q[:, i0:i0 + ih]),
            "wk": np.ascontiguousarray(w_k[:, i0:i0 + ih]),
            "wv": np.ascontiguousarray(w_v[:, i0:i0 + ih]),
            "wo": np.ascontiguousarray(w_o[i0:i0 + ih, :]),
            "bo": (b_o if hh == 0 else zeros_b).reshape(1, -1).copy(),
        })
    return in_maps


def gather_out(results, S, O, SC, n_cores=N_CORES):
    """Assemble full output from per-core ReduceScatter shards."""
    B = n_cores // 2
    out = np.empty((B, S, O), dtype=np.float32)
    nsc = S // SC
    half = SC // 2
    for core in range(n_cores):
        b, hh = core // 2, core % 2
        res = results[core]["out"]  # [S//2, O]
        for c in range(nsc):
            rows = res[c * half:(c + 1) * half]
            out[b, c * SC + hh * half: c * SC + (hh + 1) * half, :] = rows
    return out


_NC_CACHE = {}


def _get_nc():
    if "full" not in _NC_CACHE:
        _NC_CACHE["full"] = build_nc()
    return _NC_CACHE["full"]


def kernel(x, context, w_q, w_k, w_v, w_o, b_o):
    from concourse.bass_utils import run_bass_kernel_spmd

    nc = _get_nc()
    in_maps = make_in_maps(x, context, w_q, w_k, w_v, w_o, b_o)
    res = run_bass_kernel_spmd(nc, in_maps, list(range(N_CORES)))
    return gather_out(res.results, SQ_FULL, O_FULL, 512)
